# revision 15
# baseline (speedup 1.0000x reference)
"""GCN message-passing kernel for trn2 (8 NeuronCores, SPMD + AllGather).

v2 strategy:
  - Shard the N=100352 (padded) node dim across 8 cores (12544 rows each).
  - Hop h: every core gathers x[col] rows (fp16) for its edges via
    dma_gather spread over 4 SWDGE queues (parallel Q7 core pairs),
    segment-sums via PE matmuls with batched one-hot routing matrices
    (built on DVE from duplicated-pair fp16 metadata so the 16-bit 2x
    stream mode applies), then x_new = (A x) @ W + b computed row-major
    by swapping matmul operand roles (stationary=y^T, moving=W) so no
    final PE transpose is needed.
  - Pair streams: instead of gathering pair rows on device, each core
    l2-normalizes its OWN shard rows each hop and writes them out; the
    host assembles out[stream, hop, i] = xn[hop][idx[i]] (pure indexing,
    all float math on device).
  - AllGather publishes x_1 (fp16) for hop 2's gathers. No AllGather
    needed after hop 2.
"""
import os
import sys

sys.path.insert(0, "/opt/trn_rl_repo")

import numpy as np

N = 100000
D = 128
NCORES = 8
SHARD = 12544            # 98 tiles of 128
NTILE = SHARD // 128     # 98
NPAD = SHARD * NCORES    # 100352
WIN = 32768
NWIN = (NPAD + WIN - 1) // WIN  # 4
SG_TILES = 8
NSG = (NTILE + SG_TILES - 1) // SG_TILES  # 13
E_PAIR = 50000
P = 128

_CACHE = {}
LAST_RESULTS = None  # BassKernelResults of the most recent run (for test.py)


def _ceil(a, b):
    return -(-a // b)


def _pack_idx(idx_arr, cap):
    """Pack idx list (len<=cap*128, int) to the [128, cap*8] wrapped+replicated
    int16 layout. Pads with 0 (real row-0 gathers; masked by val=0)."""
    n = cap * 128
    buf = np.zeros(n, np.int16)
    buf[: len(idx_arr)] = idx_arr.astype(np.int16)
    blk = buf.reshape(n // 16, 16).T  # [16, n/16]
    return np.tile(blk, (8, 1))       # [128, n/16]


def _prep(edge_row, edge_col, edge_val):
    """Build per-core metadata + the static structure description."""
    owner = edge_row // SHARD
    per_core = []
    for c in range(NCORES):
        m = owner == c
        r = edge_row[m].astype(np.int64) - c * SHARD
        col = edge_col[m].astype(np.int64)
        val = edge_val[m]
        tile = r >> 7
        slot = r & 127
        win = col >> 15
        sg = tile // SG_TILES
        order = np.lexsort((tile, win, sg))
        per_core.append(dict(
            tile=tile[order], slot=slot[order], col=col[order],
            val=val[order], win=win[order], sg=sg[order]))

    # run partitions: key = sg*NWIN + win
    run_counts = np.zeros((NCORES, NSG * NWIN), np.int64)
    run_starts = np.zeros((NCORES, NSG * NWIN + 1), np.int64)
    for c in range(NCORES):
        d = per_core[c]
        key = d["sg"] * NWIN + d["win"]
        run_counts[c] = np.bincount(key, minlength=NSG * NWIN)
        run_starts[c, 1:] = np.cumsum(run_counts[c])

    cap_blk = np.zeros(NSG * NWIN, np.int64)
    for k in range(NSG * NWIN):
        cap_blk[k] = _ceil(int(run_counts[:, k].max()), 128)

    # per-sg gather-buffer block offsets (same layout every sg; sized by max)
    sg_bof = []
    sg_nblk = []
    for s in range(NSG):
        off = [0] * NWIN
        acc = 0
        for w in range(NWIN):
            off[w] = acc
            acc += int(cap_blk[s * NWIN + w])
        sg_bof.append(off)
        sg_nblk.append(acc)
    TOTBLK = max(sg_nblk)

    # block -> union of tiles (over cores); then tile-major MM slot list per sg
    mm_slots = []
    for s in range(NSG):
        tiles_here = list(range(s * SG_TILES, min((s + 1) * SG_TILES, NTILE)))
        cover = {}
        for w in range(NWIN):
            k = s * NWIN + w
            for b in range(int(cap_blk[k])):
                u = set()
                for c in range(NCORES):
                    st = run_starts[c, k]
                    n = run_counts[c, k]
                    lo = b * 128
                    hi = min(lo + 128, n)
                    if lo < n:
                        seg = per_core[c]["tile"][st + lo: st + hi]
                        u.update(np.unique(seg).tolist())
                cover[(w, b)] = u
        slots_s = []
        for t in tiles_here:
            for w in range(NWIN):
                for b in range(int(cap_blk[s * NWIN + w])):
                    if t in cover[(w, b)]:
                        slots_s.append((t - s * SG_TILES, w, b))
        mm_slots.append(slots_s)
    NMM = sum(len(x) for x in mm_slots)

    # per-core gidx + duplicated-pair scol2/sval2 (fp16)
    GCOLS = int(sum(cap_blk)) * 8
    gidx_arrs = []
    scol_arrs = []
    sval_arrs = []
    for c in range(NCORES):
        d = per_core[c]
        gidx = np.zeros((128, GCOLS), np.int16)
        scol2 = np.full((128, 2 * NMM), -1.0, np.float16)
        sval2 = np.zeros((128, 2 * NMM), np.float16)
        gcol_off = 0
        for s in range(NSG):
            for w in range(NWIN):
                k = s * NWIN + w
                cap = int(cap_blk[k])
                if cap == 0:
                    continue
                st, n = run_starts[c, k], run_counts[c, k]
                loc = d["col"][st: st + n] - w * WIN
                gidx[:, gcol_off: gcol_off + cap * 8] = _pack_idx(loc, cap)
                gcol_off += cap * 8
        mi = 0
        for s in range(NSG):
            for (tl, w, b) in mm_slots[s]:
                k = s * NWIN + w
                st, n = run_starts[c, k], run_counts[c, k]
                lo, hi = b * 128, min(b * 128 + 128, int(n))
                scol = np.full(128, -1.0, np.float16)
                vcol = np.zeros(128, np.float16)
                if lo < n:
                    seg_t = d["tile"][st + lo: st + hi]
                    seg_s = d["slot"][st + lo: st + hi]
                    seg_v = d["val"][st + lo: st + hi]
                    sel = seg_t == (s * SG_TILES + tl)
                    scol[: hi - lo][sel] = seg_s[sel]
                    vcol[: hi - lo][sel] = seg_v[sel].astype(np.float16)
                scol2[:, 2 * mi] = scol
                scol2[:, 2 * mi + 1] = scol
                sval2[:, 2 * mi] = vcol
                sval2[:, 2 * mi + 1] = vcol
                mi += 1
        gidx_arrs.append(gidx)
        scol_arrs.append(scol2)
        sval_arrs.append(sval2)

    structure = (
        tuple(cap_blk.tolist()),
        tuple(tuple(sl) for sg in mm_slots for sl in sg),
        tuple(len(sl) for sl in mm_slots),
        TOTBLK,
    )
    meta = dict(
        cap_blk=cap_blk, sg_bof=sg_bof, sg_nblk=sg_nblk, TOTBLK=TOTBLK,
        mm_slots=mm_slots, NMM=NMM, GCOLS=GCOLS,
        gidx_arrs=gidx_arrs, scol_arrs=scol_arrs, sval_arrs=sval_arrs,
    )
    return structure, meta


def _build_program(structure, meta):
    import concourse.bass as bass
    import concourse.mybir as mybir
    import concourse.tile as tile
    from concourse import bacc

    f16 = mybir.dt.float16
    f32 = mybir.dt.float32
    i16 = mybir.dt.int16
    AP = bass.AP

    cap_blk = meta["cap_blk"]
    sg_bof = meta["sg_bof"]
    mm_slots = meta["mm_slots"]
    NMM = meta["NMM"]
    GCOLS = meta["GCOLS"]
    TOTBLK = meta["TOTBLK"]

    nc = bacc.Bacc(None, num_devices=NCORES, num_swdge_queues=4)
    x0sh = nc.dram_tensor("x0sh", [SHARD, D], f32, kind="ExternalInput")
    x0f16 = nc.dram_tensor("x0f16", [NPAD, D], f16, kind="ExternalInput")
    gidx = nc.dram_tensor("gidx", [P, GCOLS], i16, kind="ExternalInput")
    scol2 = nc.dram_tensor("scol2", [P, 2 * NMM], f16, kind="ExternalInput")
    sval2 = nc.dram_tensor("sval2", [P, 2 * NMM], f16, kind="ExternalInput")
    w1 = nc.dram_tensor("w1", [D, D], f16, kind="ExternalInput")
    w2 = nc.dram_tensor("w2", [D, D], f16, kind="ExternalInput")
    brep1 = nc.dram_tensor("brep1", [P, D], f32, kind="ExternalInput")
    brep2 = nc.dram_tensor("brep2", [P, D], f32, kind="ExternalInput")
    xn_out = nc.dram_tensor("xn", [3, SHARD, D], f32, kind="ExternalOutput")

    # mm slot base offset per sg
    mi_base = [0] * NSG
    acc = 0
    for s in range(NSG):
        mi_base[s] = acc
        acc += len(mm_slots[s])
    # gidx col offsets per (s, w)
    gcol_off = [0] * (NSG * NWIN)
    acc = 0
    for s in range(NSG):
        for w in range(NWIN):
            gcol_off[s * NWIN + w] = acc
            acc += int(cap_blk[s * NWIN + w]) * 8

    with tile.TileContext(nc) as tc:
        with (
            tc.tile_pool(name="const", bufs=1) as cpool,
            tc.tile_pool(name="meta", bufs=1) as mpool,
            tc.tile_pool(name="gb", bufs=3) as gpool,
            tc.tile_pool(name="strip", bufs=2) as spool,
            tc.tile_pool(name="work", bufs=4) as wpool,
            tc.tile_pool(name="norm", bufs=2) as npool,
            tc.tile_pool(name="psy", bufs=4, space="PSUM") as psy,
            tc.tile_pool(name="psx", bufs=2, space="PSUM") as psx,
            tc.tile_pool(name="dram", bufs=1, space="DRAM") as dram,
        ):
            # constants
            iota_i = cpool.tile([P, P], mybir.dt.int32)
            nc.gpsimd.iota(iota_i, pattern=[[1, P]], base=0,
                           channel_multiplier=0)
            iota16 = cpool.tile([P, P], f16)
            nc.vector.tensor_copy(iota16, iota_i)
            eps_t = cpool.tile([P, 1], f32)
            nc.vector.memset(eps_t[:, :], 1e-24)
            w1_t = cpool.tile([P, P], f16)
            nc.sync.dma_start(out=w1_t, in_=w1[:, :])
            w2_t = cpool.tile([P, P], f16)
            nc.sync.dma_start(out=w2_t, in_=w2[:, :])
            b1_t = cpool.tile([P, P], f32)
            nc.sync.dma_start(out=b1_t, in_=brep1[:, :])
            b2_t = cpool.tile([P, P], f32)
            nc.sync.dma_start(out=b2_t, in_=brep2[:, :])
            gidx_t = mpool.tile([P, GCOLS], i16)
            nc.sync.dma_start(out=gidx_t, in_=gidx[:, :])
            scol_t = mpool.tile([P, 2 * NMM], f16)
            nc.sync.dma_start(out=scol_t, in_=scol2[:, :])
            sval_t = mpool.tile([P, 2 * NMM], f16)
            nc.sync.dma_start(out=sval_t, in_=sval2[:, :])

            # internal DRAM
            xsh1 = dram.tile([SHARD, D], f16)
            xg1 = dram.tile([NPAD, D], f16, addr_space="Shared")

            def strip_aps(strip, strip2, m0, S):
                """4-D packed-pair APs for the batched one-hot build."""
                st = strip[:, :, :]
                st4 = AP(st.tensor, st.offset,
                         [st.ap[0], [128, S], [2, 64], [1, 2]])
                st2 = strip2[:, :, :]
                st24 = AP(st2.tensor, st2.offset,
                          [st2.ap[0], [128, S], [2, 64], [1, 2]])
                io = iota16[:, :]
                io4 = AP(io.tensor, io.offset,
                         [io.ap[0], [0, S], [2, 64], [1, 2]])
                sc = scol_t[:, 2 * m0: 2 * (m0 + S)]
                sc4 = AP(sc.tensor, sc.offset,
                         [sc.ap[0], [2, S], [0, 64], [1, 2]])
                sv = sval_t[:, 2 * m0: 2 * (m0 + S)]
                sv4 = AP(sv.tensor, sv.offset,
                         [sv.ap[0], [2, S], [0, 64], [1, 2]])
                return st4, st24, io4, sc4, sv4

            def norm_rows(xin, ntl, dst_ap, rdt):
                """l2-normalize rows of xin [P, ntl, P] -> dst (f32)."""
                sq = npool.tile([P, SG_TILES, P], f32, tag="sq")
                nc.vector.tensor_tensor(
                    out=sq[:, :ntl, :], in0=xin[:, :ntl, :],
                    in1=xin[:, :ntl, :], op=mybir.AluOpType.mult)
                rs = npool.tile([P, SG_TILES], f32, tag="rs")
                nc.vector.tensor_reduce(
                    out=rs[:, :ntl], in_=sq[:, :ntl, :],
                    axis=mybir.AxisListType.X, op=mybir.AluOpType.add)
                nrm = npool.tile([P, SG_TILES], f32, tag="nrm")
                nc.scalar.activation(nrm[:, :ntl], rs[:, :ntl],
                                     mybir.ActivationFunctionType.Sqrt,
                                     bias=eps_t[:, :1])
                rinv = npool.tile([P, SG_TILES], rdt, tag="rinv")
                with nc.allow_low_precision(reason="f16 rinv; 5e-4 ok"):
                    nc.vector.reciprocal(rinv[:, :ntl], nrm[:, :ntl])
                xo = npool.tile([P, SG_TILES, P], f32, tag="xo")
                ri = rinv[:, :ntl]
                ri_b = AP(ri.tensor, ri.offset, [ri.ap[0], [1, ntl], [0, P]])
                nc.vector.tensor_tensor(
                    out=xo[:, :ntl, :], in0=xin[:, :ntl, :], in1=ri_b,
                    op=mybir.AluOpType.mult)
                nc.sync.dma_start(
                    out=dst_ap.rearrange("(c p) d -> p c d", p=P),
                    in_=xo[:, :ntl, :])

            def pair0_stage():
                """hop-0: l2norm own shard rows from exact f32 input."""
                for s in range(NSG):
                    ntl = min(SG_TILES, NTILE - s * SG_TILES)
                    r0 = s * SG_TILES * 128
                    x0t = npool.tile([P, SG_TILES, P], f32, tag="x0t")
                    nc.sync.dma_start(
                        out=x0t[:, :ntl, :],
                        in_=x0sh[r0: r0 + ntl * 128, :].rearrange(
                            "(c p) d -> p c d", p=P))
                    norm_rows(x0t, ntl, xn_out[0, r0: r0 + ntl * 128, :],
                              f32)

            def graph_hop(src, w_t, b_t, xsh, hop):
                """One GCN hop: x_new = (A @ src) @ W + b; write l2norm of
                own-shard rows to xn_out[hop]; optionally publish xsh."""
                qload = [0, 0, 0, 0]
                for s in range(NSG):
                    gbuf = gpool.tile([P, TOTBLK, P], f16, tag="gbuf")
                    CH = 11
                    for w in range(NWIN):
                        k = s * NWIN + w
                        cap = int(cap_blk[k])
                        if cap == 0:
                            continue
                        hi = min(NPAD, (w + 1) * WIN)
                        for lo in range(0, cap, CH):
                            ln = min(CH, cap - lo)
                            q = qload.index(min(qload))
                            qload[q] += ln
                            bo = sg_bof[s][w] + lo
                            co = gcol_off[k] + lo * 8
                            nc.gpsimd.dma_gather(
                                gbuf[:, bo: bo + ln, :],
                                src[w * WIN: hi, :],
                                gidx_t[:, co: co + ln * 8],
                                num_idxs=ln * 128, num_idxs_reg=ln * 128,
                                elem_size=P, single_packet=False,
                                queue_num=q,
                            )
                    slots = mm_slots[s]
                    ntl = min(SG_TILES, NTILE - s * SG_TILES)
                    xrows = npool.tile([P, SG_TILES, P], f16, tag="xrows")
                    for t in range(ntl):
                        tslots = [(i, sl) for i, sl in enumerate(slots)
                                  if sl[0] == t]
                        S = len(tslots)
                        m0 = mi_base[s] + tslots[0][0]
                        strip = spool.tile([P, S, P], f16, tag="strip")
                        strip2 = spool.tile([P, S, P], f16, tag="strip2")
                        st4, st24, io4, sc4, sv4 = strip_aps(
                            strip, strip2, m0, S)
                        nc.vector.tensor_tensor(
                            out=st4, in0=io4, in1=sc4,
                            op=mybir.AluOpType.is_equal)
                        nc.vector.tensor_tensor(
                            out=st24, in0=st4, in1=sv4,
                            op=mybir.AluOpType.mult)
                        y_ps = psy.tile([P, P], f32, space="PSUM", tag="y")
                        for si, (i, (tl, w, b)) in enumerate(tslots):
                            gb = sg_bof[s][w] + b
                            nc.tensor.matmul(
                                y_ps, lhsT=gbuf[:, gb, :],
                                rhs=strip2[:, si, :],
                                start=(si == 0), stop=(si == S - 1),
                            )
                        yT = wpool.tile([P, P], f16, tag="yT")
                        nc.scalar.copy(yT, y_ps)
                        x_ps = psx.tile([P, P], f32, space="PSUM", tag="x")
                        nc.tensor.matmul(x_ps, lhsT=yT, rhs=w_t,
                                         start=True, stop=True)
                        nc.vector.tensor_tensor(
                            out=xrows[:, t, :], in0=x_ps[:, :], in1=b_t[:, :],
                            op=mybir.AluOpType.add)
                    r0 = s * SG_TILES * 128
                    if xsh is not None:
                        nc.sync.dma_start(
                            out=xsh[r0: r0 + ntl * 128, :].rearrange(
                                "(c p) d -> p c d", p=P),
                            in_=xrows[:, :ntl, :])
                    norm_rows(xrows, ntl,
                              xn_out[hop, r0: r0 + ntl * 128, :], f16)

            stages = os.environ.get(
                "BASS_GNN_STAGES", "p0,h1,ag1,h2").split(",")
            if "h1" in stages:
                graph_hop(x0f16, w1_t, b1_t, xsh1, 1)
            if "ag1" in stages:
                nc.gpsimd.collective_compute(
                    "AllGather", mybir.AluOpType.bypass,
                    replica_groups=[list(range(NCORES))],
                    ins=[xsh1.opt()], outs=[xg1.opt()],
                )
            if "p0" in stages:
                pair0_stage()
            if "h2" in stages:
                graph_hop(xg1, w2_t, b2_t, None, 2)

    nc.compile()
    return nc


def _install_ntff_shim():
    """Provide antenv.axon_hooks (missing on this image) so trace=True can
    capture NTFF profiles through the axon .so."""
    import types
    if "antenv.axon_hooks" in sys.modules:
        return
    mod = types.ModuleType("antenv.axon_hooks")
    mod._hook = None

    def set_axon_ntff_profile_hook(h):
        mod._hook = h

    def get_axon_ntff_profile_hook():
        return mod._hook

    mod.set_axon_ntff_profile_hook = set_axon_ntff_profile_hook
    mod.get_axon_ntff_profile_hook = get_axon_ntff_profile_hook
    sys.modules["antenv.axon_hooks"] = mod
    try:
        from trn_agent_boot.trn_boot import _ntff_profile_via_ctypes
        mod._hook = _ntff_profile_via_ctypes("/opt/axon/libaxon_pjrt.so")
    except Exception:
        mod._hook = None


def kernel(node_emb, attri_emb, W1, b1, W2, b2, edge_val,
           edge_row, edge_col, pos_src, pos_dst, neg_src, neg_dst):
    global LAST_RESULTS
    _install_ntff_shim()
    from concourse.bass_utils import run_bass_kernel_spmd

    structure, meta = _prep(edge_row, edge_col, edge_val)

    import time as _time
    key = (structure, os.environ.get("BASS_GNN_STAGES", ""))
    if key in _CACHE:
        nc = _CACHE[key]
    else:
        t0 = _time.time()
        nc = _build_program(structure, meta)
        print(f"[kernel] build+schedule: {_time.time() - t0:.1f}s, "
              f"{len(nc.inst_map)} instructions", flush=True)
        _CACHE[key] = nc

    x0 = np.concatenate([node_emb, attri_emb], axis=0).astype(np.float32)
    x0p = np.zeros((NPAD, D), np.float32)
    x0p[:N] = x0
    x0p16 = x0p.astype(np.float16)

    in_maps = []
    for c in range(NCORES):
        in_maps.append({
            "x0sh": x0p[c * SHARD: (c + 1) * SHARD],
            "x0f16": x0p16,
            "gidx": meta["gidx_arrs"][c],
            "scol2": meta["scol_arrs"][c],
            "sval2": meta["sval_arrs"][c],
            "w1": W1.astype(np.float16),
            "w2": W2.astype(np.float16),
            "brep1": np.broadcast_to(
                b1.astype(np.float32)[None, :], (P, D)).copy(),
            "brep2": np.broadcast_to(
                b2.astype(np.float32)[None, :], (P, D)).copy(),
        })

    trace = os.environ.get("BASS_GNN_TRACE", "0") == "1"
    t0 = _time.time()
    res = run_bass_kernel_spmd(nc, in_maps, core_ids=list(range(NCORES)),
                               trace=trace)
    print(f"[kernel] compile+run: {_time.time() - t0:.1f}s", flush=True)
    LAST_RESULTS = res

    # ---- host assembly: index normalized tables per hop ----
    xn_full = np.empty((3, NPAD, D), np.float32)
    for c in range(NCORES):
        xn_full[:, c * SHARD: (c + 1) * SHARD] = res.results[c]["xn"]
    out = np.empty((4, 3, E_PAIR, D), np.float32)
    for st, idx in enumerate((pos_src, pos_dst, neg_src, neg_dst)):
        idx64 = idx.astype(np.int64)
        for h in range(3):
            out[st, h] = xn_full[h, idx64]
    return out


# revision 16
# speedup vs baseline: 1.0305x; 1.0305x over previous
"""GCN message-passing kernel for trn2 (8 NeuronCores, SPMD + split AllGather).

v5 strategy:
  - Shard the N=100352 (padded) node dim across 8 cores (12544 rows each).
  - Hop h: every core gathers x[col] rows (fp16) for its edges via
    dma_gather spread over 4 SWDGE queues (parallel Q7 core pairs),
    segment-sums via PE matmuls with batched one-hot routing matrices
    (built on DVE from duplicated-pair fp16 metadata so the 16-bit 2x
    stream mode applies), then x_new = (A x) @ W + b computed row-major
    by swapping matmul operand roles (stationary=y^T, moving=W).
  - Pair streams: each core l2-normalizes its OWN shard rows each hop and
    writes them out; the host assembles out[s, h, i] = xn[h][idx[i]].
  - The x_1 AllGather is split into two sg-aligned halves (tiles 0-47 /
    48-97). AGa is triggered mid-hop-1 so its entry barrier (core skew)
    and data movement hide behind hop-1 compute; only AGb's tail is
    exposed. Hop 2 uses half-interleaved gather metadata.
"""
import os
import sys

sys.path.insert(0, "/opt/trn_rl_repo")

import numpy as np

N = 100000
D = 128
NCORES = 8
SHARD = 12544            # 98 tiles of 128
NTILE = SHARD // 128     # 98
NPAD = SHARD * NCORES    # 100352
WIN = 32768
NWIN = 4
SG_TILES = 8
NSG = (NTILE + SG_TILES - 1) // SG_TILES  # 13
HA_SG = 6                # sgs 0-5 -> half A
HA = HA_SG * SG_TILES * 128   # 6144 rows
HB = SHARD - HA               # 6400 rows
NA = HA * NCORES              # 49152
NB = HB * NCORES              # 51200
E_PAIR = 50000
P = 128

_CACHE = {}
LAST_RESULTS = None  # BassKernelResults of the most recent run (for test.py)


def _ceil(a, b):
    return -(-a // b)


def _pack_idx(idx_arr, cap):
    """Pack idx list (len<=cap*128, int) to the [128, cap*8] wrapped+replicated
    int16 layout. Pads with 0 (real row-0 gathers; masked by val=0)."""
    n = cap * 128
    buf = np.zeros(n, np.int16)
    buf[: len(idx_arr)] = idx_arr.astype(np.int16)
    blk = buf.reshape(n // 16, 16).T  # [16, n/16]
    return np.tile(blk, (8, 1))       # [128, n/16]


def _hop_meta(cores):
    """Per-hop metadata from per-core dicts {tile, slot, loc, win, val}.
    Edges are re-sorted by (sg, win, tile); runs are keyed (sg, win)."""
    per_core = []
    for d in cores:
        sg = d["tile"] // SG_TILES
        order = np.lexsort((d["tile"], d["win"], sg))
        per_core.append(dict(
            tile=d["tile"][order], slot=d["slot"][order],
            loc=d["loc"][order], val=d["val"][order],
            win=d["win"][order], sg=sg[order]))

    run_counts = np.zeros((NCORES, NSG * NWIN), np.int64)
    run_starts = np.zeros((NCORES, NSG * NWIN + 1), np.int64)
    for c in range(NCORES):
        d = per_core[c]
        key = d["sg"] * NWIN + d["win"]
        run_counts[c] = np.bincount(key, minlength=NSG * NWIN)
        run_starts[c, 1:] = np.cumsum(run_counts[c])

    cap_blk = np.zeros(NSG * NWIN, np.int64)
    for k in range(NSG * NWIN):
        cap_blk[k] = _ceil(int(run_counts[:, k].max()), 128)

    sg_bof = []
    sg_nblk = []
    for s in range(NSG):
        off = [0] * NWIN
        acc = 0
        for w in range(NWIN):
            off[w] = acc
            acc += int(cap_blk[s * NWIN + w])
        sg_bof.append(off)
        sg_nblk.append(acc)
    TOTBLK = max(sg_nblk)

    mm_slots = []
    for s in range(NSG):
        tiles_here = list(range(s * SG_TILES, min((s + 1) * SG_TILES, NTILE)))
        cover = {}
        for w in range(NWIN):
            k = s * NWIN + w
            for b in range(int(cap_blk[k])):
                u = set()
                for c in range(NCORES):
                    st = run_starts[c, k]
                    n = run_counts[c, k]
                    lo = b * 128
                    hi = min(lo + 128, n)
                    if lo < n:
                        seg = per_core[c]["tile"][st + lo: st + hi]
                        u.update(np.unique(seg).tolist())
                cover[(w, b)] = u
        slots_s = []
        for t in tiles_here:
            for w in range(NWIN):
                for b in range(int(cap_blk[s * NWIN + w])):
                    if t in cover[(w, b)]:
                        slots_s.append((t - s * SG_TILES, w, b))
        mm_slots.append(slots_s)
    NMM = sum(len(x) for x in mm_slots)

    GCOLS = int(sum(cap_blk)) * 8
    gidx_arrs = []
    scol_arrs = []
    sval_arrs = []
    for c in range(NCORES):
        d = per_core[c]
        gidx = np.zeros((128, GCOLS), np.int16)
        scol2 = np.full((128, 2 * NMM), -1.0, np.float16)
        sval2 = np.zeros((128, 2 * NMM), np.float16)
        gcol = 0
        for s in range(NSG):
            for w in range(NWIN):
                k = s * NWIN + w
                cap = int(cap_blk[k])
                if cap == 0:
                    continue
                st, n = run_starts[c, k], run_counts[c, k]
                loc = d["loc"][st: st + n]
                gidx[:, gcol: gcol + cap * 8] = _pack_idx(loc, cap)
                gcol += cap * 8
        mi = 0
        for s in range(NSG):
            for (tl, w, b) in mm_slots[s]:
                k = s * NWIN + w
                st, n = run_starts[c, k], run_counts[c, k]
                lo, hi = b * 128, min(b * 128 + 128, int(n))
                scol = np.full(128, -1.0, np.float16)
                vcol = np.zeros(128, np.float16)
                if lo < n:
                    seg_t = d["tile"][st + lo: st + hi]
                    seg_s = d["slot"][st + lo: st + hi]
                    seg_v = d["val"][st + lo: st + hi]
                    sel = seg_t == (s * SG_TILES + tl)
                    scol[: hi - lo][sel] = seg_s[sel]
                    vcol[: hi - lo][sel] = seg_v[sel].astype(np.float16)
                scol2[:, 2 * mi] = scol
                scol2[:, 2 * mi + 1] = scol
                sval2[:, 2 * mi] = vcol
                sval2[:, 2 * mi + 1] = vcol
                mi += 1
        gidx_arrs.append(gidx)
        scol_arrs.append(scol2)
        sval_arrs.append(sval2)

    return dict(
        cap_blk=cap_blk, sg_bof=sg_bof, TOTBLK=TOTBLK,
        mm_slots=mm_slots, NMM=NMM, GCOLS=GCOLS,
        gidx_arrs=gidx_arrs, scol_arrs=scol_arrs, sval_arrs=sval_arrs,
        structure=(tuple(cap_blk.tolist()),
                   tuple(tuple(sl) for sg in mm_slots for sl in sg),
                   tuple(len(sl) for sl in mm_slots),
                   TOTBLK),
    )


def _prep(edge_row, edge_col, edge_val):
    owner = edge_row // SHARD
    cores1 = []
    cores2 = []
    for c in range(NCORES):
        m = owner == c
        r = edge_row[m].astype(np.int64) - c * SHARD
        col = edge_col[m].astype(np.int64)
        val = edge_val[m]
        tile = r >> 7
        slot = r & 127
        # hop 1: plain row-major x0 layout
        cores1.append(dict(tile=tile, slot=slot, val=val,
                           win=col >> 15, loc=col & 32767))
        # hop 2: half-interleaved xg1a/xg1b layout
        c2 = col // SHARD
        rr = col % SHARD
        in_a = rr < HA
        local = np.where(in_a, c2 * HA + rr, c2 * HB + (rr - HA))
        win2 = np.where(in_a, local >> 15, 2 + (local >> 15)).astype(np.int64)
        cores2.append(dict(tile=tile, slot=slot, val=val,
                           win=win2, loc=local & 32767))
    mA = _hop_meta(cores1)
    mB = _hop_meta(cores2)
    structure = (mA["structure"], mB["structure"])
    return structure, dict(h1=mA, h2=mB)


def _build_program(structure, meta):
    import concourse.bass as bass
    import concourse.mybir as mybir
    import concourse.tile as tile
    from concourse import bacc

    f16 = mybir.dt.float16
    f32 = mybir.dt.float32
    i16 = mybir.dt.int16
    AP = bass.AP

    mA, mB = meta["h1"], meta["h2"]
    GX = max(mA["GCOLS"], mB["GCOLS"])
    NM = max(mA["NMM"], mB["NMM"])
    TOTBLK = max(mA["TOTBLK"], mB["TOTBLK"])

    nc = bacc.Bacc(None, num_devices=NCORES, num_swdge_queues=4)
    x0sh = nc.dram_tensor("x0sh", [SHARD, D], f32, kind="ExternalInput")
    x0f16 = nc.dram_tensor("x0f16", [NPAD, D], f16, kind="ExternalInput")
    gidxA = nc.dram_tensor("gidxA", [P, mA["GCOLS"]], i16, kind="ExternalInput")
    scolA = nc.dram_tensor("scolA", [P, 2 * mA["NMM"]], f16, kind="ExternalInput")
    svalA = nc.dram_tensor("svalA", [P, 2 * mA["NMM"]], f16, kind="ExternalInput")
    gidxB = nc.dram_tensor("gidxB", [P, mB["GCOLS"]], i16, kind="ExternalInput")
    scolB = nc.dram_tensor("scolB", [P, 2 * mB["NMM"]], f16, kind="ExternalInput")
    svalB = nc.dram_tensor("svalB", [P, 2 * mB["NMM"]], f16, kind="ExternalInput")
    w1 = nc.dram_tensor("w1", [D, D], f16, kind="ExternalInput")
    w2 = nc.dram_tensor("w2", [D, D], f16, kind="ExternalInput")
    brep1 = nc.dram_tensor("brep1", [P, D], f32, kind="ExternalInput")
    brep2 = nc.dram_tensor("brep2", [P, D], f32, kind="ExternalInput")
    xn_out = nc.dram_tensor("xn", [3, SHARD, D], f32, kind="ExternalOutput")

    def offs(m):
        mi_base = [0] * NSG
        acc = 0
        for s in range(NSG):
            mi_base[s] = acc
            acc += len(m["mm_slots"][s])
        gcol_off = [0] * (NSG * NWIN)
        acc = 0
        for s in range(NSG):
            for w in range(NWIN):
                gcol_off[s * NWIN + w] = acc
                acc += int(m["cap_blk"][s * NWIN + w]) * 8
        return mi_base, gcol_off

    miA, gcoA = offs(mA)
    miB, gcoB = offs(mB)

    with tile.TileContext(nc) as tc:
        with (
            tc.tile_pool(name="const", bufs=1) as cpool,
            tc.tile_pool(name="meta", bufs=1) as mpool,
            tc.tile_pool(name="gb", bufs=3) as gpool,
            tc.tile_pool(name="strip", bufs=2) as spool,
            tc.tile_pool(name="work", bufs=4) as wpool,
            tc.tile_pool(name="norm", bufs=2) as npool,
            tc.tile_pool(name="psy", bufs=4, space="PSUM") as psy,
            tc.tile_pool(name="psx", bufs=2, space="PSUM") as psx,
            tc.tile_pool(name="dram", bufs=1, space="DRAM") as dram,
        ):
            # constants
            iota_i = cpool.tile([P, P], mybir.dt.int32)
            nc.gpsimd.iota(iota_i, pattern=[[1, P]], base=0,
                           channel_multiplier=0)
            iota16 = cpool.tile([P, P], f16)
            nc.vector.tensor_copy(iota16, iota_i)
            eps_t = cpool.tile([P, 1], f32)
            nc.vector.memset(eps_t[:, :], 1e-24)
            w1_t = cpool.tile([P, P], f16)
            nc.sync.dma_start(out=w1_t, in_=w1[:, :])
            w2_t = cpool.tile([P, P], f16)
            nc.sync.dma_start(out=w2_t, in_=w2[:, :])
            b1_t = cpool.tile([P, P], f32)
            nc.sync.dma_start(out=b1_t, in_=brep1[:, :])
            b2_t = cpool.tile([P, P], f32)
            nc.sync.dma_start(out=b2_t, in_=brep2[:, :])
            gidx_t = mpool.tile([P, GX], i16)
            nc.sync.dma_start(out=gidx_t[:, : mA["GCOLS"]], in_=gidxA[:, :])
            scol_t = mpool.tile([P, 2 * NM], f16)
            nc.sync.dma_start(out=scol_t[:, : 2 * mA["NMM"]], in_=scolA[:, :])
            sval_t = mpool.tile([P, 2 * NM], f16)
            nc.sync.dma_start(out=sval_t[:, : 2 * mA["NMM"]], in_=svalA[:, :])

            # internal DRAM
            xsh1a = dram.tile([HA, D], f16)
            xsh1b = dram.tile([HB, D], f16)
            xg1a = dram.tile([NA, D], f16, addr_space="Shared")
            xg1b = dram.tile([NB, D], f16, addr_space="Shared")

            def strip_aps(strip, strip2, m0, S):
                st = strip[:, :, :]
                st4 = AP(st.tensor, st.offset,
                         [st.ap[0], [128, S], [2, 64], [1, 2]])
                st2 = strip2[:, :, :]
                st24 = AP(st2.tensor, st2.offset,
                          [st2.ap[0], [128, S], [2, 64], [1, 2]])
                io = iota16[:, :]
                io4 = AP(io.tensor, io.offset,
                         [io.ap[0], [0, S], [2, 64], [1, 2]])
                sc = scol_t[:, 2 * m0: 2 * (m0 + S)]
                sc4 = AP(sc.tensor, sc.offset,
                         [sc.ap[0], [2, S], [0, 64], [1, 2]])
                sv = sval_t[:, 2 * m0: 2 * (m0 + S)]
                sv4 = AP(sv.tensor, sv.offset,
                         [sv.ap[0], [2, S], [0, 64], [1, 2]])
                return st4, st24, io4, sc4, sv4

            def norm_rows(xin, ntl, dst_ap, rdt):
                sq = npool.tile([P, SG_TILES, P], f32, tag="sq")
                nc.vector.tensor_tensor(
                    out=sq[:, :ntl, :], in0=xin[:, :ntl, :],
                    in1=xin[:, :ntl, :], op=mybir.AluOpType.mult)
                rs = npool.tile([P, SG_TILES], f32, tag="rs")
                nc.vector.tensor_reduce(
                    out=rs[:, :ntl], in_=sq[:, :ntl, :],
                    axis=mybir.AxisListType.X, op=mybir.AluOpType.add)
                nrm = npool.tile([P, SG_TILES], f32, tag="nrm")
                nc.scalar.activation(nrm[:, :ntl], rs[:, :ntl],
                                     mybir.ActivationFunctionType.Sqrt,
                                     bias=eps_t[:, :1])
                rinv = npool.tile([P, SG_TILES], rdt, tag="rinv")
                with nc.allow_low_precision(reason="f16 rinv; 5e-4 ok"):
                    nc.vector.reciprocal(rinv[:, :ntl], nrm[:, :ntl])
                xo = npool.tile([P, SG_TILES, P], f32, tag="xo")
                ri = rinv[:, :ntl]
                ri_b = AP(ri.tensor, ri.offset, [ri.ap[0], [1, ntl], [0, P]])
                nc.vector.tensor_tensor(
                    out=xo[:, :ntl, :], in0=xin[:, :ntl, :], in1=ri_b,
                    op=mybir.AluOpType.mult)
                nc.sync.dma_start(
                    out=dst_ap.rearrange("(c p) d -> p c d", p=P),
                    in_=xo[:, :ntl, :])

            def pair0_stage():
                for s in range(NSG):
                    ntl = min(SG_TILES, NTILE - s * SG_TILES)
                    r0 = s * SG_TILES * 128
                    x0t = npool.tile([P, SG_TILES, P], f32, tag="x0t")
                    nc.sync.dma_start(
                        out=x0t[:, :ntl, :],
                        in_=x0sh[r0: r0 + ntl * 128, :].rearrange(
                            "(c p) d -> p c d", p=P))
                    norm_rows(x0t, ntl, xn_out[0, r0: r0 + ntl * 128, :], f32)

            def emit_ag(half):
                if half == 0:
                    nc.gpsimd.collective_compute(
                        "AllGather", mybir.AluOpType.bypass,
                        replica_groups=[list(range(NCORES))],
                        ins=[xsh1a.opt()], outs=[xg1a.opt()],
                    )
                else:
                    nc.gpsimd.collective_compute(
                        "AllGather", mybir.AluOpType.bypass,
                        replica_groups=[list(range(NCORES))],
                        ins=[xsh1b.opt()], outs=[xg1b.opt()],
                    )

            def graph_hop(m, mi_base, gcol_off, srcs, w_t, b_t, xsh_fn,
                          hop, post_gather=None):
                cap_blk = m["cap_blk"]
                sg_bof = m["sg_bof"]
                mm_slots = m["mm_slots"]
                qload = [0, 0, 0, 0]
                for s in range(NSG):
                    gbuf = gpool.tile([P, TOTBLK, P], f16, tag="gbuf")
                    CH = 11
                    for w in range(NWIN):
                        k = s * NWIN + w
                        cap = int(cap_blk[k])
                        if cap == 0:
                            continue
                        for lo in range(0, cap, CH):
                            ln = min(CH, cap - lo)
                            q = qload.index(min(qload))
                            qload[q] += ln
                            bo = sg_bof[s][w] + lo
                            co = gcol_off[k] + lo * 8
                            nc.gpsimd.dma_gather(
                                gbuf[:, bo: bo + ln, :],
                                srcs[w],
                                gidx_t[:, co: co + ln * 8],
                                num_idxs=ln * 128, num_idxs_reg=ln * 128,
                                elem_size=P, single_packet=False,
                                queue_num=q,
                            )
                    if post_gather is not None:
                        post_gather(s)
                    slots = mm_slots[s]
                    ntl = min(SG_TILES, NTILE - s * SG_TILES)
                    xrows = npool.tile([P, SG_TILES, P], f16, tag="xrows")
                    for t in range(ntl):
                        tslots = [(i, sl) for i, sl in enumerate(slots)
                                  if sl[0] == t]
                        S = len(tslots)
                        m0 = mi_base[s] + tslots[0][0]
                        strip = spool.tile([P, S, P], f16, tag="strip")
                        strip2 = spool.tile([P, S, P], f16, tag="strip2")
                        st4, st24, io4, sc4, sv4 = strip_aps(
                            strip, strip2, m0, S)
                        nc.vector.tensor_tensor(
                            out=st4, in0=io4, in1=sc4,
                            op=mybir.AluOpType.is_equal)
                        nc.vector.tensor_tensor(
                            out=st24, in0=st4, in1=sv4,
                            op=mybir.AluOpType.mult)
                        y_ps = psy.tile([P, P], f32, space="PSUM", tag="y")
                        for si, (i, (tl, w, b)) in enumerate(tslots):
                            gb = sg_bof[s][w] + b
                            nc.tensor.matmul(
                                y_ps, lhsT=gbuf[:, gb, :],
                                rhs=strip2[:, si, :],
                                start=(si == 0), stop=(si == S - 1),
                            )
                        yT = wpool.tile([P, P], f16, tag="yT")
                        nc.scalar.copy(yT, y_ps)
                        x_ps = psx.tile([P, P], f32, space="PSUM", tag="x")
                        nc.tensor.matmul(x_ps, lhsT=yT, rhs=w_t,
                                         start=True, stop=True)
                        nc.vector.tensor_tensor(
                            out=xrows[:, t, :], in0=x_ps[:, :], in1=b_t[:, :],
                            op=mybir.AluOpType.add)
                    r0 = s * SG_TILES * 128
                    if xsh_fn is not None:
                        nc.sync.dma_start(
                            out=xsh_fn(s, ntl).rearrange(
                                "(c p) d -> p c d", p=P),
                            in_=xrows[:, :ntl, :])
                    norm_rows(xrows, ntl,
                              xn_out[hop, r0: r0 + ntl * 128, :], f16)

            def xsh1_fn(s, ntl):
                if s < HA_SG:
                    r = s * SG_TILES * 128
                    return xsh1a[r: r + ntl * 128, :]
                r = (s - HA_SG) * SG_TILES * 128
                return xsh1b[r: r + ntl * 128, :]

            srcs1 = [x0f16[w * WIN: min(NPAD, (w + 1) * WIN), :]
                     for w in range(NWIN)]
            srcs2 = [xg1a[0: WIN, :], xg1a[WIN: NA, :],
                     xg1b[0: WIN, :], xg1b[WIN: NB, :]]

            def h1_hook(s):
                # trigger the half-A AllGather once its inputs exist and
                # the wait will be satisfied at dispatch (sg 7 > 5)
                if s == 7:
                    emit_ag(0)

            stages = os.environ.get(
                "BASS_GNN_STAGES", "p0,h1,ag1,h2").split(",")
            if "h1" in stages:
                graph_hop(mA, miA, gcoA, srcs1, w1_t, b1_t, xsh1_fn, 1,
                          post_gather=h1_hook)
            if "ag1" in stages:
                emit_ag(1)
            # overwrite metadata SBUF with hop-2 tables (Tile orders these
            # after hop-1's last reads)
            nc.sync.dma_start(out=gidx_t[:, : mB["GCOLS"]], in_=gidxB[:, :])
            nc.sync.dma_start(out=scol_t[:, : 2 * mB["NMM"]], in_=scolB[:, :])
            nc.sync.dma_start(out=sval_t[:, : 2 * mB["NMM"]], in_=svalB[:, :])
            if "p0" in stages:
                pair0_stage()
            if "h2" in stages:
                graph_hop(mB, miB, gcoB, srcs2, w2_t, b2_t, None, 2)

    nc.compile()
    return nc


def _install_ntff_shim():
    """Provide antenv.axon_hooks (missing on this image) so trace=True can
    capture NTFF profiles through the axon .so."""
    import types
    if "antenv.axon_hooks" in sys.modules:
        return
    mod = types.ModuleType("antenv.axon_hooks")
    mod._hook = None

    def set_axon_ntff_profile_hook(h):
        mod._hook = h

    def get_axon_ntff_profile_hook():
        return mod._hook

    mod.set_axon_ntff_profile_hook = set_axon_ntff_profile_hook
    mod.get_axon_ntff_profile_hook = get_axon_ntff_profile_hook
    sys.modules["antenv.axon_hooks"] = mod
    try:
        from trn_agent_boot.trn_boot import _ntff_profile_via_ctypes
        mod._hook = _ntff_profile_via_ctypes("/opt/axon/libaxon_pjrt.so")
    except Exception:
        mod._hook = None


def kernel(node_emb, attri_emb, W1, b1, W2, b2, edge_val,
           edge_row, edge_col, pos_src, pos_dst, neg_src, neg_dst):
    global LAST_RESULTS
    _install_ntff_shim()
    from concourse.bass_utils import run_bass_kernel_spmd

    structure, meta = _prep(edge_row, edge_col, edge_val)

    import time as _time
    key = (structure, os.environ.get("BASS_GNN_STAGES", ""))
    if key in _CACHE:
        nc = _CACHE[key]
    else:
        t0 = _time.time()
        nc = _build_program(structure, meta)
        print(f"[kernel] build+schedule: {_time.time() - t0:.1f}s, "
              f"{len(nc.inst_map)} instructions", flush=True)
        _CACHE[key] = nc

    x0 = np.concatenate([node_emb, attri_emb], axis=0).astype(np.float32)
    x0p = np.zeros((NPAD, D), np.float32)
    x0p[:N] = x0
    x0p16 = x0p.astype(np.float16)

    mA, mB = meta["h1"], meta["h2"]
    in_maps = []
    for c in range(NCORES):
        in_maps.append({
            "x0sh": x0p[c * SHARD: (c + 1) * SHARD],
            "x0f16": x0p16,
            "gidxA": mA["gidx_arrs"][c],
            "scolA": mA["scol_arrs"][c],
            "svalA": mA["sval_arrs"][c],
            "gidxB": mB["gidx_arrs"][c],
            "scolB": mB["scol_arrs"][c],
            "svalB": mB["sval_arrs"][c],
            "w1": W1.astype(np.float16),
            "w2": W2.astype(np.float16),
            "brep1": np.broadcast_to(
                b1.astype(np.float32)[None, :], (P, D)).copy(),
            "brep2": np.broadcast_to(
                b2.astype(np.float32)[None, :], (P, D)).copy(),
        })

    trace = os.environ.get("BASS_GNN_TRACE", "0") == "1"
    t0 = _time.time()
    res = run_bass_kernel_spmd(nc, in_maps, core_ids=list(range(NCORES)),
                               trace=trace)
    print(f"[kernel] compile+run: {_time.time() - t0:.1f}s", flush=True)
    LAST_RESULTS = res

    # ---- host assembly: index normalized tables per hop ----
    xn_full = np.empty((3, NPAD, D), np.float32)
    for c in range(NCORES):
        xn_full[:, c * SHARD: (c + 1) * SHARD] = res.results[c]["xn"]
    out = np.empty((4, 3, E_PAIR, D), np.float32)
    for st, idx in enumerate((pos_src, pos_dst, neg_src, neg_dst)):
        idx64 = idx.astype(np.int64)
        for h in range(3):
            out[st, h] = xn_full[h, idx64]
    return out
